# revision 67
# baseline (speedup 1.0000x reference)
"""MixerBlock TRN2 kernel: B=2, S=4096, E=1024, DF=4096 on 8 NeuronCores.

Strategy (two SPMD launches):
  Phase 1 (shard B*S=8192 rows -> 1024 rows/core):
    h   = LN(x)            (cn affine folded into W1/b1 host-side)
    a   = silu(h @ W1g + b1')        -> kept transposed aT[df, tok]
    y   = x + aT.T @ W2 + b2
    h2  = LN(y)*tn_g + tn_b          (bf16)
    outputs y (f32), h2 (bf16)
  Phase 2 (shard E=1024 -> 128 channels/core; rows (b,e) = 256/core):
    out[be, s] = sum_t h2T[t, be] * M[t, s] + tb[s] + y[be, s]
    The Toeplitz matrix M[t,s] = tw[s-t] (s>=t) is diagonal-constant, so a
    [128t x 512s] tile depends only on (512*sb - 128*t): 32 distinct tiles,
    prebuilt host-side from tw (4 MB bf16), used as the moving operand.
"""

import os
import sys

sys.path.insert(0, "/opt/trn_rl_repo")
sys.path.insert(0, "/opt/trn_rl_repo/concourse")

import numpy as np
import ml_dtypes

import concourse.bass as bass
import concourse.bacc as bacc
import concourse.mybir as mybir
from concourse import tile
from concourse import bass_utils
from concourse.bass_interp import get_hw_module

dt = mybir.dt
AF = mybir.ActivationFunctionType
AX = mybir.AxisListType
BF16 = ml_dtypes.bfloat16

B, S, E = 2, 4096, 1024
DF = 4 * E
EPS = 1e-5
NCORES = 8
RPC = (B * S) // NCORES      # 1024 rows per core (phase 1)
EPC = E // NCORES            # 128 channels per core (phase 2)
BE = B * EPC                 # 256 (b,e) rows per core (phase 2)

LAST_TIMINGS = {}

# --------------------------------------------------------------------------
# phase 1 program
# --------------------------------------------------------------------------


def build_phase1():
    nc = bacc.Bacc("TRN2", target_bir_lowering=False, debug=False,
                   enable_asserts=False, num_devices=NCORES)
    x_d = nc.dram_tensor("x", [RPC, E], dt.float32, kind="ExternalInput").ap()
    xb_d = nc.dram_tensor("xb", [RPC, E], dt.float32, kind="ExternalInput").ap()
    w1_d = nc.dram_tensor("w1", [E, DF], dt.bfloat16, kind="ExternalInput").ap()
    # W2 pre-scaled by 64, fp8 e4m3, laid out [p, d, e] = W2[128d+p, e]
    w2q_d = nc.dram_tensor("w2q", [128, 32, E], dt.float8e4, kind="ExternalInput").ap()
    b1_d = nc.dram_tensor("b1", [128, 32], dt.float32, kind="ExternalInput").ap()
    id_d = nc.dram_tensor("ident", [128, 128], dt.bfloat16, kind="ExternalInput").ap()
    y_d = nc.dram_tensor("y", [RPC, E], dt.float32, kind="ExternalOutput").ap()
    st_d = nc.dram_tensor("st", [RPC, 2], dt.float32, kind="ExternalOutput").ap()

    NT = 4          # token tiles per block (block = 512 tokens)
    NBLK = RPC // (128 * NT)   # 2 blocks
    TW = 128 * NT   # token width per block

    from contextlib import ExitStack
    with tile.TileContext(nc) as tc, ExitStack() as es:
        pool = lambda **kw: es.enter_context(tc.tile_pool(**kw))
        constp = pool(name="const", bufs=1)
        w1p = pool(name="w1p", bufs=8)
        xp = pool(name="xp", bufs=5)
        xrp = pool(name="xrp", bufs=6)
        statp = pool(name="stat", bufs=24)
        hbfp = pool(name="hbf", bufs=3)
        htp = pool(name="htp", bufs=17)
        atp = pool(name="atp", bufs=20)
        w2p = pool(name="w2p", bufs=4)
        yp = pool(name="yp", bufs=5)
        mps = pool(name="mps", bufs=8, space="PSUM")
        if True:
            junk = constp.tile([128, 512], dt.bfloat16, tag="junk")
            nc.vector.memset(junk[:, :], 0.25)
            id_sb = constp.tile([128, 128], dt.bfloat16, tag="ident")
            nc.sync.dma_start(out=id_sb[:, :], in_=id_d[:, :])
            eps_sb = constp.tile([128, 1], dt.float32, tag="eps")
            nc.gpsimd.memset(eps_sb[:, :], EPS)
            # preload ACT tables (sqrt/silu) while DMAs land
            t11 = constp.tile([128, 1], dt.float32, tag="t11")
            nc.gpsimd.memset(t11[:, :], 1.0)
            nc.scalar.activation(t11[:, :], t11[:, :], AF.Sqrt,
                                 scale=1.0, bias=eps_sb[:, :])
            nc.scalar.activation(t11[:, :], t11[:, :], AF.Silu,
                                 bias=eps_sb[:, :])
            # HAM warmup: dense dummy matmuls while first x tiles load
            wps = mps.tile([128, 512], dt.float32, tag="mp", name="warm")
            for i in range(20):
                nc.tensor.matmul(wps[:, :], junk[:, 0:128], junk[:, :],
                                 start=(i == 0), stop=(i == 19))

            def ln_stats(srct):
                """returns mv [128,2] = (mean, rstd) of rows of srct."""
                stats = statp.tile([128, 2, 6], dt.float32, tag="bst")
                for i in range(2):
                    nc.vector.bn_stats(stats[:, i, :],
                                       srct[:, i * 512:(i + 1) * 512])
                mv = statp.tile([128, 2], dt.float32, tag="mv")
                nc.vector.bn_aggr(mv[:, :], stats[:, :, :])
                nc.scalar.activation(mv[:, 1:2], mv[:, 1:2], AF.Sqrt,
                                     scale=1.0, bias=eps_sb[:, :])
                nc.vector.reciprocal(mv[:, 1:2], mv[:, 1:2])
                return mv

            hT = [[None] * 8 for _ in range(NBLK)]
            xts = [[None] * NT for _ in range(NBLK)]

            def load_x(blk, tt):
                row0 = blk * 128 * NT
                xt = xp.tile([128, E], dt.float32, tag="xt",
                             name=f"xt{blk}_{tt}")
                nc.sync.dma_start(
                    out=xt[:, :],
                    in_=x_d[row0 + tt * 128: row0 + (tt + 1) * 128, :])
                xts[blk][tt] = xt

            def ln_transpose_tile(blk, tt, bridge):
                xt = xts[blk][tt]
                mv = ln_stats(xt)
                hb = hbfp.tile([128, E], dt.bfloat16, tag="hb",
                               name=f"hb{blk}_{tt}")
                nc.vector.tensor_scalar(hb[:, :], xt[:, :],
                                        mv[:, 0:1], mv[:, 1:2],
                                        op0=mybir.AluOpType.subtract,
                                        op1=mybir.AluOpType.mult)
                for e in range(8):
                    pt = mps.tile([128, 128], dt.bfloat16, tag="mp",
                                  name=f"tp{blk}_{tt}_{e}")
                    nc.tensor.transpose(
                        pt[:, :], hb[:, e * 128:(e + 1) * 128], id_sb[:, :])
                    if hT[blk][e] is None:
                        hT[blk][e] = htp.tile([128, TW], dt.bfloat16,
                                              tag="ht", name=f"ht{blk}_{e}")
                    if e % 2 == 0:
                        nc.vector.tensor_copy(
                            hT[blk][e][:, tt * 128:(tt + 1) * 128], pt[:, :])
                    else:
                        nc.scalar.copy(
                            hT[blk][e][:, tt * 128:(tt + 1) * 128], pt[:, :])
                if bridge:
                    # keep the PE warm while the next LN chain completes
                    bps = mps.tile([128, 512], dt.float32, tag="mp",
                                   name=f"bridge{blk}_{tt}")
                    for i in range(6):
                        nc.tensor.matmul(bps[:, :], junk[:, 0:128], junk[:, :],
                                         start=(i == 0), stop=(i == 5))

            # all bulk DMA issues on sync (sole issuer), consumption order
            def load_xr(blk):
                for tt in range(NT):
                    xr = xrp.tile([128, E], dt.float32, tag="xr",
                                  name=f"xr{blk}_{tt}")
                    nc.sync.dma_start(
                        out=xr[:, :],
                        in_=xb_d[blk * TW + tt * 128:
                                 blk * TW + (tt + 1) * 128, :])
                    xr_t[blk][tt] = xr

            xr_t = [[None] * NT for _ in range(NBLK)]
            load_x(0, 0)
            load_x(0, 1)
            load_x(0, 2)
            load_x(0, 3)
            w1f = []
            for e in range(8):
                t = w1p.tile([128, DF], dt.bfloat16, tag="w1sb",
                             name=f"w1_{e}")
                nc.sync.dma_start(out=t[:, :],
                                  in_=w1_d[e * 128:(e + 1) * 128, :])
                w1f.append(t)
            ln_transpose_tile(0, 0, bridge=True)
            ln_transpose_tile(0, 1, bridge=True)
            ln_transpose_tile(0, 2, bridge=True)
            ln_transpose_tile(0, 3, bridge=True)
            b1_sb = constp.tile([128, 32], dt.float32, tag="b1")
            nc.sync.dma_start(out=b1_sb[:, :], in_=b1_d[:, :])
            for tt in range(NT):
                load_x(1, tt)
            load_xr(0)
            # resident fp8 W2, 4 tiles for fine-grained readiness
            w2t4 = []
            for j in range(4):
                t = w2p.tile([128, 8, E], dt.float8e4, tag="w2q",
                             name=f"w2q{j}")
                nc.sync.dma_start(out=t[:, :, :],
                                  in_=w2q_d[:, 8 * j:8 * (j + 1), :])
                w2t4.append(t)
            load_xr(1)

            for blk in range(NBLK):
                row0 = blk * TW
                # ---- mm1 + silu -> aTp[j][128 df-pair, 2, TW tok] (fp8) ----
                # e-major inside df-blocks of 8 rides the w1 DMA wavefront
                aTp = [None] * 16
                for dfb in range(4):
                    pss1 = []
                    for k in range(8):
                        ps = mps.tile([128, 512], dt.float32, tag="mp",
                                      name=f"m1_{blk}_{dfb}_{k}")
                        pss1.append(ps)
                    for e in range(8):
                        for k in range(8):
                            c0 = dfb * 1024 + k * 128
                            nc.tensor.matmul(
                                pss1[k][:, 0:TW],
                                w1f[e][:, c0:c0 + 128],
                                hT[blk][e][:, :],
                                start=(e == 0), stop=(e == 7))
                    for k in range(8):
                        df = dfb * 8 + k
                        if df % 2 == 0:
                            aTp[df // 2] = atp.tile([128, 2, TW],
                                                    dt.float8e4, tag="at",
                                                    name=f"at{blk}_{df // 2}")
                        nc.scalar.activation(aTp[df // 2][:, df % 2, :],
                                             pss1[k][:, 0:TW], AF.Silu,
                                             bias=b1_sb[:, df:df + 1])
                if blk + 1 < NBLK:
                    # next block's LN ran on DVE during this mm1; transposes
                    # execute here back-to-back
                    for tt in range(NT):
                        ln_transpose_tile(blk + 1, tt, bridge=False)

                # ---- mm2 (fp8 DoubleRow, W2 resident) ----
                # last block goes eb-major across tts so the final drains
                # and y stores spread out instead of backloading the tail
                y_ts = {}
                stats_ts = {}

                def mm2_group(tt, eb):
                    ps = mps.tile([128, 512], dt.float32, tag="mp",
                                  name=f"m2_{blk}_{tt}_{eb}")
                    for j in range(16):
                        nc.tensor.matmul(
                            ps[:, :],
                            aTp[j][:, :, tt * 128:(tt + 1) * 128],
                            w2t4[j // 4][:, 2 * (j % 4):2 * (j % 4) + 2,
                                         eb * 512:(eb + 1) * 512],
                            start=(j == 0), stop=(j == 15),
                            perf_mode=mybir.MatmulPerfMode.DoubleRow)
                    if eb == 0:
                        y_ts[tt] = yp.tile([128, E], dt.float32, tag="yt",
                                           name=f"yt{blk}_{tt}")
                        stats_ts[tt] = statp.tile([128, 2, 6], dt.float32,
                                                  tag="bst",
                                                  name=f"yst{blk}_{tt}")
                    y_t, stats = y_ts[tt], stats_ts[tt]
                    nc.vector.scalar_tensor_tensor(
                        y_t[:, eb * 512:(eb + 1) * 512],
                        ps[:, :], 1.0 / 64.0,
                        xr_t[blk][tt][:, eb * 512:(eb + 1) * 512],
                        op0=mybir.AluOpType.mult,
                        op1=mybir.AluOpType.add)
                    eng = nc.gpsimd if eb == 0 else nc.sync
                    eng.dma_start(
                        out=y_d[row0 + tt * 128: row0 + (tt + 1) * 128,
                                eb * 512:(eb + 1) * 512],
                        in_=y_t[:, eb * 512:(eb + 1) * 512])
                    nc.vector.bn_stats(stats[:, eb, :],
                                       y_t[:, eb * 512:(eb + 1) * 512])
                    if eb == 1:
                        mv2 = statp.tile([128, 2], dt.float32, tag="mv",
                                         name=f"ymv{blk}_{tt}")
                        nc.vector.bn_aggr(mv2[:, :], stats[:, :, :])
                        nc.scalar.activation(mv2[:, 1:2], mv2[:, 1:2],
                                             AF.Sqrt,
                                             scale=1.0, bias=eps_sb[:, :])
                        nc.vector.reciprocal(mv2[:, 1:2], mv2[:, 1:2])
                        nc.gpsimd.dma_start(
                            out=st_d[row0 + tt * 128:
                                     row0 + (tt + 1) * 128, :],
                            in_=mv2[:, :])

                order = ([(tt, eb) for tt in range(NT) for eb in range(2)]
                         if blk + 1 < NBLK else
                         [(tt, 0) for tt in range(NT)]
                         + [(tt, 1) for tt in range(NT)])
                for tt, eb in order:
                    mm2_group(tt, eb)
    nc.compile()
    nc.m = get_hw_module(nc.m)
    return nc


# --------------------------------------------------------------------------
# phase 2 program
# --------------------------------------------------------------------------


def build_phase2():
    nc = bacc.Bacc("TRN2", target_bir_lowering=False, debug=False,
                   enable_asserts=False, num_devices=NCORES)
    # packed layouts: y2_d[p, t*BE + be] = yT[t*128+p, be]  (bf16)
    #                 r_d[p, d, j] = tw[128d + j - p]  (0 outside [0,S))
    #                 stp_d[p, 4t+2b+k] = (-mean*rstd, rstd) of row (b, t*128+p)
    #                 yt_d[s, be] = y + tb + tn_b*cumsum(tw)  (bias pre-folded)
    y2_d = nc.dram_tensor("y2", [128, 32 * BE], dt.bfloat16, kind="ExternalInput").ap()
    r_d = nc.dram_tensor("rt", [128, 32, 128], dt.bfloat16, kind="ExternalInput").ap()
    stp_d = nc.dram_tensor("stp", [128, 128], dt.float32, kind="ExternalInput").ap()
    yt_d = nc.dram_tensor("yt", [S, BE], dt.float32, kind="ExternalInput").ap()
    g2_d = nc.dram_tensor("g2", [128, BE], dt.float32, kind="ExternalInput").ap()
    out_d = nc.dram_tensor("out", [S, BE], dt.float32, kind="ExternalOutput").ap()

    from contextlib import ExitStack
    with tile.TileContext(nc) as tc, ExitStack() as es:
        pool = lambda **kw: es.enter_context(tc.tile_pool(**kw))
        y2p = pool(name="y2", bufs=4)
        hsp = pool(name="hs", bufs=32)
        rtp = pool(name="rt", bufs=4)
        constp = pool(name="const", bufs=1)
        yinp = pool(name="yin", bufs=8)
        outp = pool(name="outp", bufs=8)
        psp = pool(name="ps", bufs=7, space="PSUM")
        warmp = pool(name="warm", bufs=1, space="PSUM")
        if True:
            # warmup while the first chunks load
            junk = constp.tile([128, 512], dt.bfloat16, tag="junk")
            nc.gpsimd.memset(junk[:, :], 0.25)
            wps = warmp.tile([128, 512], dt.float32, tag="warm", name="warm")
            for i in range(9):
                nc.tensor.matmul(wps[:, :], junk[:, 0:128], junk[:, :],
                                 start=(i == 0), stop=(i == 8))

            # loads in consumption order; early chunks split for fast landing
            y2_t = [None] * 4   # [128, 2048] each (8 t-tiles)
            rt_t = [None] * 4   # [128, 8, 128] each (8 d-tiles)

            def load_rt(c, nsplit=1):
                rt_t[c] = rtp.tile([128, 8, 128], dt.bfloat16, tag="rt",
                                   name=f"rt{c}")
                w = 8 // nsplit
                for k in range(nsplit):
                    nc.sync.dma_start(
                        out=rt_t[c][:, k * w:(k + 1) * w, :],
                        in_=r_d[:, c * 8 + k * w: c * 8 + (k + 1) * w, :])

            def load_y2(c, nsplit=1):
                y2_t[c] = y2p.tile([128, 2048], dt.bfloat16, tag="y2",
                                   name=f"y2{c}")
                w = 2048 // nsplit
                for k in range(nsplit):
                    nc.sync.dma_start(
                        out=y2_t[c][:, k * w:(k + 1) * w],
                        in_=y2_d[:, c * 2048 + k * w: c * 2048 + (k + 1) * w])

            stp_sb = constp.tile([128, 128], dt.float32, tag="stp")
            nc.sync.dma_start(out=stp_sb[:, :], in_=stp_d[:, :])
            y2_t[0] = y2p.tile([128, 2048], dt.bfloat16, tag="y2", name="y20")
            rt_t[0] = rtp.tile([128, 8, 128], dt.bfloat16, tag="rt",
                               name="rt0")
            nc.sync.dma_start(out=y2_t[0][:, 0:1024], in_=y2_d[:, 0:1024])
            nc.sync.dma_start(out=rt_t[0][:, 0:4, :], in_=r_d[:, 0:4, :])
            nc.sync.dma_start(out=y2_t[0][:, 1024:2048],
                              in_=y2_d[:, 1024:2048])
            nc.sync.dma_start(out=rt_t[0][:, 4:8, :], in_=r_d[:, 4:8, :])
            g2_sb = constp.tile([128, BE], dt.float32, tag="g2")
            nc.sync.dma_start(out=g2_sb[:, :], in_=g2_d[:, :])
            load_rt(1)
            load_y2(1)
            load_rt(2)
            load_y2(2)
            load_rt(3)
            load_y2(3)

            # normalize on ACT just-in-time: hs[t] half = y2*rstd + (-mean*rstd)
            hs = [None] * 32

            def make_hs(t):
                hs[t] = hsp.tile([128, BE], dt.bfloat16, tag="hs",
                                 name=f"hs{t}")
                for b in range(2):
                    c0 = 4 * t + 2 * b
                    nc.scalar.activation(
                        hs[t][:, b * 128:(b + 1) * 128],
                        y2_t[t // 8][:, (t % 8) * BE + b * 128:
                                     (t % 8) * BE + (b + 1) * 128],
                        AF.Identity,
                        scale=stp_sb[:, c0 + 1:c0 + 2],
                        bias=stp_sb[:, c0:c0 + 1])

            for t in range(4):
                make_hs(t)
            # out[sg*128+j, be] = sum_t hs[t][:, be]^T rt[sg-t]  (+ g, resid)
            for sg in range(32):
                if sg + 4 < 32:
                    make_hs(sg + 4)
                psf = psp.tile([128, 512], dt.float32, tag="ps",
                               name=f"ps{sg}")
                for t in range(sg + 1):
                    d = sg - t
                    nc.tensor.matmul(
                        psf[:, 0:BE], rt_t[d // 8][:, d % 8, :], hs[t][:, :],
                        start=(t == 0), stop=(t == sg))
                yin = yinp.tile([128, BE], dt.float32, tag="yin")
                nc.sync.dma_start(
                    out=yin[:, :],
                    in_=yt_d[sg * 128:(sg + 1) * 128, :])
                gp = outp.tile([128, BE], dt.float32, tag="gp", name=f"gp{sg}")
                nc.vector.tensor_mul(gp[:, :], psf[:, 0:BE], g2_sb[:, :])
                ot = outp.tile([128, BE], dt.float32, tag="ot", name=f"ot{sg}")
                nc.vector.tensor_add(ot[:, :], gp[:, :], yin[:, :])
                dma_eng = nc.gpsimd if sg % 2 == 0 else nc.scalar
                dma_eng.dma_start(
                    out=out_d[sg * 128:(sg + 1) * 128, :], in_=ot[:, :])
    nc.compile()
    nc.m = get_hw_module(nc.m)
    return nc


def _install_ntff_hook():
    """The agent image's antenv lacks axon_hooks; synthesize it so
    run_bass_kernel_spmd(trace=True) can capture NTFF profiles."""
    import types
    import antenv

    if "antenv.axon_hooks" in sys.modules:
        return
    mod = types.ModuleType("antenv.axon_hooks")
    state = {"h": None}
    mod.set_axon_ntff_profile_hook = lambda h: state.__setitem__("h", h)
    mod.get_axon_ntff_profile_hook = lambda: state["h"]
    sys.modules["antenv.axon_hooks"] = mod
    antenv.axon_hooks = mod
    from trn_agent_boot.trn_boot import _ntff_profile_via_ctypes

    mod.set_axon_ntff_profile_hook(
        _ntff_profile_via_ctypes("/opt/axon/libaxon_pjrt.so"))
    bass_utils.upload_artifacts = lambda tmpdir: tmpdir


_P1 = None
_P2 = None


def _programs():
    global _P1, _P2
    if _P1 is None:
        _P1 = build_phase1()
    if _P2 is None:
        _P2 = build_phase2()
    return _P1, _P2


def _run(nc, in_maps, trace):
    if trace:
        try:
            _install_ntff_hook()
        except Exception as e:
            print(f"ntff hook install failed: {e}", file=sys.stderr)
            trace = False
    res = bass_utils.run_bass_kernel_spmd(
        nc, in_maps, core_ids=list(range(NCORES)), trace=trace)
    return res


def kernel(x, cn_g, cn_b, W1, b1, W2, b2, tn_g, tn_b, tw, tb):
    trace = os.environ.get("MIXER_TRACE", "0") == "1"
    x = np.asarray(x, np.float32)
    p1, p2 = _programs()

    # ---- host prep (inputs only) ----
    W1 = np.asarray(W1, np.float32)
    W2 = np.asarray(W2, np.float32)
    cn_g = np.asarray(cn_g, np.float32)
    cn_b = np.asarray(cn_b, np.float32)
    w1g = (cn_g[:, None] * W1).astype(BF16)
    b1f = (np.asarray(b1, np.float32) + cn_b @ W1).astype(np.float32)
    b1_t = np.ascontiguousarray(b1f.reshape(32, 128).T)          # [128, 32]
    # fp8 W2 (x64 prescale), [p, d, e] = 64*W2[128d+p, e]
    w2q = np.ascontiguousarray(
        (W2 * 64.0).reshape(32, 128, E).transpose(1, 0, 2)
        .astype(ml_dtypes.float8_e4m3))
    xbf = (x + np.asarray(b2, np.float32)).reshape(B * S, E)     # x + b2
    ident = np.eye(128, dtype=BF16)
    tn_g = np.asarray(tn_g, np.float32)
    tn_b = np.asarray(tn_b, np.float32)

    xf = x.reshape(B * S, E)
    in_maps1 = []
    for c in range(NCORES):
        in_maps1.append({
            "x": np.ascontiguousarray(xf[c * RPC:(c + 1) * RPC]),
            "xb": np.ascontiguousarray(xbf[c * RPC:(c + 1) * RPC]),
            "w1": w1g, "w2q": w2q, "b1": b1_t, "ident": ident,
        })
    r1 = _run(p1, in_maps1, trace)
    if trace:
        LAST_TIMINGS["phase1_ns"] = r1.exec_time_ns
    y = np.concatenate([np.asarray(r1.results[c]["y"], np.float32)
                        for c in range(NCORES)], axis=0)
    st = np.concatenate([np.asarray(r1.results[c]["st"], np.float32)
                         for c in range(NCORES)], axis=0)       # [B*S, 2]

    # ---- phase 2 host glue ----
    tw = np.asarray(tw, np.float32)
    tb_f = np.asarray(tb, np.float32)
    # rt[p, d, j] = tw[128d + j - p]  (0 outside [0, S))
    pad = np.zeros(128 + S + 128, np.float32)
    pad[128:128 + S] = tw
    win = np.lib.stride_tricks.sliding_window_view(pad, 128)   # win[k] = pad[k:k+128]
    R = np.empty((32, 128, 128), np.float32)
    ii = np.arange(128)
    for d in range(32):
        R[d] = win[128 + 128 * d - ii]
    rt_np = np.ascontiguousarray(
        R.transpose(1, 0, 2).astype(BF16))                     # [128, 32, 128]
    cums = np.cumsum(tw)

    # per-(b,token) LN2 stats packed [128, 128]: stp[p, 4t+2b+k] = stv[b, t*128+p, k]
    stv = st.reshape(B, S, 2)
    stm = np.stack([-stv[..., 0] * stv[..., 1], stv[..., 1]], axis=-1)
    stp = np.ascontiguousarray(
        stm.reshape(2, 32, 128, 2).transpose(2, 1, 0, 3).reshape(128, 128))
    yv = y.reshape(B, S, E)
    in_maps2 = []
    for c in range(NCORES):
        e0 = c * EPC
        ysl_bt = yv[:, :, e0:e0 + EPC]
        y2sl = np.ascontiguousarray(
            ysl_bt.transpose(1, 0, 2).astype(BF16).reshape(32, 128, BE)
            .transpose(1, 0, 2).reshape(128, 32 * BE))
        g = tn_g[e0:e0 + EPC]
        tnb = tn_b[e0:e0 + EPC]
        bias = np.outer(cums, tnb) + tb_f[:, None]             # [S, 128]
        yin = np.concatenate([ysl_bt[0] + bias, ysl_bt[1] + bias], axis=1)
        g2 = np.broadcast_to(np.concatenate([g, g])[None, :], (128, BE))
        in_maps2.append({
            "y2": y2sl, "rt": rt_np, "stp": stp,
            "yt": np.ascontiguousarray(yin, dtype=np.float32),
            "g2": np.ascontiguousarray(g2, dtype=np.float32)})
    r2 = _run(p2, in_maps2, trace)
    if trace:
        LAST_TIMINGS["phase2_ns"] = r2.exec_time_ns

    out = np.empty((B, S, E), np.float32)
    for c in range(NCORES):
        e0 = c * EPC
        o = np.asarray(r2.results[c]["out"], np.float32)       # [S, BE]
        out[0, :, e0:e0 + EPC] = o[:, :EPC]
        out[1, :, e0:e0 + EPC] = o[:, EPC:]
    return out

